# revision 32
# baseline (speedup 1.0000x reference)
"""Grouped MLP (MoE expert FFN) Bass kernel for 8 Trainium2 NeuronCores.

Problem: 4096 tokens sorted by expert (8 experts, uneven counts), per-expert
GLU MLP:  h = x @ w1[g]  (-> up|gate, 2*2048 cols);  a = silu(up)*gate;
y = a @ w2[g].

Sharding: 8-way tensor-parallel over the intermediate dim (perfectly
load-balanced for ANY token counts).  Core c holds, for every expert g:
  w1 slice  [1024, 512] = up cols [c*256,(c+1)*256) | gate cols same range
  w2 slice  [256, 1024] = rows [c*256,(c+1)*256)
Every core processes ALL tokens (in per-expert chunks; the chunk->expert
table is compile-time constant and identical on all cores -> one SPMD
program) and emits a bf16 partial y; the host sums the 8 partials in fp32.

All tensors are bf16 on device (1 PE cycle/row like f32r, but half the HBM
traffic); PSUM accumulation is fp32.  DMAs are few and large with
partition-major DRAM layouts (2-8KB contiguous lines) so each transfer
spreads across all 16 SDMA engines.
"""

import sys

try:  # concourse normally comes from the container's PYTHONPATH
    import concourse  # noqa: F401
except ImportError:  # pragma: no cover - fallback for stripped env
    for _p in (
        "/root/.axon_site",
        "/root/.axon_site/_ro/trn_rl_repo",
        "/root/.axon_site/_ro/pypackages",
        "/opt/trn_rl_repo",
    ):
        if _p not in sys.path:
            sys.path.append(_p)

from contextlib import ExitStack

import numpy as np

NUM_TOKENS = 4096
HIDDEN = 1024
INTER = 2048
GROUPS = 8
N_CORES = 8
ISL = INTER // N_CORES  # 256: per-core inter slice (per up/gate half)
KB1 = HIDDEN // 128  # 8 fc1 k-blocks
KB2 = ISL // 128  # 2 fc2 k-blocks
MO = HIDDEN // 128  # 8 fc2 out row-blocks

_PROGRAM_CACHE: dict = {}


def _chunks_from_counts(counts):
    """Per-expert token chunks (off, nl, g); nl <= 512, 128-aligned splits.

    Experts are ordered by descending token count so early compute covers
    the DMA time of the remaining weight loads (chunk offsets stay in the
    token-sorted coordinate system; only processing order changes).
    """
    chunks = []
    offs = np.zeros(len(counts) + 1, np.int64)
    offs[1:] = np.cumsum(counts)
    order = sorted(range(len(counts)), key=lambda g: -int(counts[g]))
    for gi, g in enumerate(order):
        cnt = int(counts[g])
        off = int(offs[g])
        seg = 0
        if gi == 0 and cnt >= 512:
            chunks.append((off, 256, g))
            seg = 256
        while seg < cnt:
            rem = cnt - seg
            if rem > 512:
                # balance the split: 768 -> 384+384, 640 -> 384+256
                n = -(-rem // 512)
                nl = min(512, ((rem // n + 127) // 128) * 128)
            else:
                nl = rem
            chunks.append((off + seg, nl, g))
            seg += nl
    return tuple(chunks)


def _build_program(chunks, total):
    """Build + compile the single-core Bass program (same NEFF on all cores)."""
    import concourse.bass as bass  # noqa: F401
    import concourse.mybir as mybir
    import concourse.tile as tile
    from concourse import bacc

    f32 = mybir.dt.float32
    bf16 = mybir.dt.bfloat16
    silu = mybir.ActivationFunctionType.Silu

    nc = bacc.Bacc("TRN2", target_bir_lowering=False, debug=False)

    xc_d = nc.dram_tensor("xc", [128, KB1 * total], bf16, kind="ExternalInput").ap()
    w1_d = nc.dram_tensor(
        "w1c", [GROUPS, 128, KB1 * 2 * ISL], bf16, kind="ExternalInput"
    ).ap()
    w2_d = nc.dram_tensor(
        "w2c", [GROUPS, 128, KB2 * HIDDEN], bf16, kind="ExternalInput"
    ).ap()
    y_d = nc.dram_tensor("yc", [128, MO * total], bf16, kind="ExternalOutput").ap()

    expert_order = []
    for _, _, g in chunks:
        if g not in expert_order:
            expert_order.append(g)

    with tile.TileContext(nc) as tc, ExitStack() as ctx:
        wp = ctx.enter_context(tc.tile_pool(name="w", bufs=1))
        xp = ctx.enter_context(tc.tile_pool(name="x", bufs=6))
        hp = ctx.enter_context(tc.tile_pool(name="h", bufs=3))
        yp = ctx.enter_context(tc.tile_pool(name="y", bufs=3))
        tp = ctx.enter_context(tc.tile_pool(name="t", bufs=3))
        pup = ctx.enter_context(tc.tile_pool(name="pu", bufs=3, space="PSUM"))
        pgp = ctx.enter_context(tc.tile_pool(name="pg", bufs=2, space="PSUM"))
        pyp = ctx.enter_context(tc.tile_pool(name="py", bufs=3, space="PSUM"))

        # Warm-up matmuls: keep the PE busy from t=0 so the p-state ramps to
        # 2.4GHz while the first weight/token DMAs are still in flight.
        # Two alternating PSUM tiles so consecutive warm-ups pipeline.
        wu = wp.tile([128, 128], bf16, tag="warm")
        nc.gpsimd.memset(wu, 0)
        pwa = pyp.tile([128, 128], f32, tag="py", name="pwa")
        pwb = pyp.tile([128, 128], f32, tag="py", name="pwb")
        for i in range(112):
            nc.tensor.matmul(
                pwa if i % 2 == 0 else pwb, wu, wu, start=True, stop=True
            )

        # All weight loads go upfront on the Act HWDGE queue (they are never
        # slot-gated); token/output traffic runs on the SP queue so the two
        # streams cannot block each other.
        w1t = {}
        w2t = {}

        def load_weights(g):
            w1t[g] = wp.tile(
                [128, KB1 * 2 * ISL], bf16, tag=f"w1g{g}", name=f"w1t{g}"
            )
            nc.scalar.dma_start(out=w1t[g], in_=w1_d[g])
            w2t[g] = wp.tile(
                [128, KB2 * HIDDEN], bf16, tag=f"w2g{g}", name=f"w2t{g}"
            )
            nc.scalar.dma_start(out=w2t[g], in_=w2_d[g])

        load_weights(expert_order[0])

        def emit_fc2(off, nl, g, ht, final=False):
            """fc2 + psum->sbuf downcast + y store for a finished fc1 chunk."""
            yt = yp.tile([128, MO * nl], bf16, tag="y", name="yt")
            for mo in range(MO):
                py = pyp.tile([128, nl], f32, tag="py", name="py")
                for kb in range(KB2):
                    nc.tensor.matmul(
                        py,
                        w2t[g][:, kb * HIDDEN + mo * 128 : kb * HIDDEN + mo * 128 + 128],
                        ht[:, kb * nl : kb * nl + nl],
                        start=(kb == 0),
                        stop=(kb == KB2 - 1),
                    )
                dst = yt[:, mo * nl : mo * nl + nl]
                if mo % 2 == 0:
                    nc.scalar.copy(dst, py)
                else:
                    nc.vector.tensor_copy(dst, py)
                if mo == 3:
                    # first half leaves while mo 4-7 are still being copied;
                    # also shortens the in-order SP sequencer's wait (the
                    # next x-load issue sits behind this store issue)
                    nc.sync.dma_start(
                        out=y_d[:, MO * off : MO * off + 4 * nl],
                        in_=yt[:, : 4 * nl],
                    )
            nc.sync.dma_start(
                out=y_d[:, MO * off + 4 * nl : MO * off + MO * nl],
                in_=yt[:, 4 * nl :],
            )

        # Software-pipelined by one chunk: fc2 of chunk c-1 is emitted after
        # fc1 of chunk c, so fc2 never waits on the silu/mul chain and the
        # next chunk's silu precedes the previous chunk's copies on ACT.
        pending = None
        for off, nl, g in chunks:
            xt = xp.tile([128, KB1 * nl], bf16, tag="x")
            nc.sync.dma_start(
                out=xt, in_=xc_d[:, KB1 * off : KB1 * off + KB1 * nl]
            )
            # prefetch the next expert's weights one expert ahead; spreading
            # the 667ns ACT-sequencer issue slots across chunks keeps them
            # from ever blocking a pending silu
            gi = expert_order.index(g)
            if gi + 1 < len(expert_order) and expert_order[gi + 1] not in w1t:
                load_weights(expert_order[gi + 1])
            ht = hp.tile([128, KB2 * nl], bf16, tag="h")
            for i2 in range(KB2):
                pu = pup.tile([128, nl], f32, tag="pu")
                pg = pgp.tile([128, nl], f32, tag="pg")
                for kb in range(KB1):
                    rhs = xt[:, kb * nl : kb * nl + nl]
                    w = w1t[g][:, kb * 2 * ISL + i2 * 128 :]
                    nc.tensor.matmul(
                        pu, w[:, :128], rhs, start=(kb == 0), stop=(kb == KB1 - 1)
                    )
                for kb in range(KB1):
                    rhs = xt[:, kb * nl : kb * nl + nl]
                    w = w1t[g][:, kb * 2 * ISL + ISL + i2 * 128 :]
                    nc.tensor.matmul(
                        pg, w[:, :128], rhs, start=(kb == 0), stop=(kb == KB1 - 1)
                    )
                tmp = tp.tile([128, nl], f32, tag="tmp")
                nc.scalar.activation(tmp, pu, silu)
                nc.vector.tensor_mul(ht[:, i2 * nl : i2 * nl + nl], tmp, pg)
            if pending is not None:
                emit_fc2(*pending)
            pending = (off, nl, g, ht)
        emit_fc2(*pending, final=True)

    nc.compile()
    return nc


def _get_program(chunks, total):
    key = (chunks, total)
    if key not in _PROGRAM_CACHE:
        _PROGRAM_CACHE[key] = _build_program(chunks, total)
    return _PROGRAM_CACHE[key]


_LAST_RESULTS = {}  # exposed for test.py (exec time, trace paths)


def kernel(permuted_tokens, tokens_per_expert, w1, w2, _trace=False):
    import ml_dtypes
    from concourse.bass_utils import run_bass_kernel_spmd

    bf16 = ml_dtypes.bfloat16

    x = np.asarray(permuted_tokens, np.float32)
    counts = np.asarray(tokens_per_expert, np.int64)
    w1 = np.asarray(w1, np.float32)
    w2 = np.asarray(w2, np.float32)
    T = x.shape[0]

    offs = np.zeros(GROUPS + 1, np.int64)
    offs[1:] = np.cumsum(counts)

    # pad each expert segment to a multiple of 128 (no-op for 128-aligned counts)
    pcounts = [(-(-int(c) // 128)) * 128 for c in counts]
    total = sum(pcounts)
    poffs = np.zeros(GROUPS + 1, np.int64)
    poffs[1:] = np.cumsum(pcounts)
    if total == T and all(int(a) == b for a, b in zip(counts, pcounts)):
        xp = x
    else:
        xp = np.zeros((total, HIDDEN), np.float32)
        for g in range(GROUPS):
            xp[poffs[g] : poffs[g] + int(counts[g])] = x[offs[g] : offs[g + 1]]

    chunks = _chunks_from_counts(pcounts)
    nc = _get_program(chunks, total)

    # ---- host-side packing (shared token tensor; per-core weight slices) ----
    xcast = xp.astype(bf16)
    xc = np.empty((128, KB1 * total), bf16)
    for off, nl, _ in chunks:
        seg = xcast[off : off + nl].T.reshape(KB1, 128, nl).transpose(1, 0, 2)
        xc[:, KB1 * off : KB1 * (off + nl)] = seg.reshape(128, KB1 * nl)

    w1b = w1.astype(bf16)
    w2b = w2.astype(bf16)

    in_maps = []
    for c in range(N_CORES):
        w1c = np.empty((GROUPS, 128, KB1 * 2 * ISL), bf16)
        w2c = np.empty((GROUPS, 128, KB2 * HIDDEN), bf16)
        lo = c * ISL
        for g in range(GROUPS):
            s = np.concatenate(
                [w1b[g][:, lo : lo + ISL], w1b[g][:, INTER + lo : INTER + lo + ISL]],
                axis=1,
            )  # [1024, 512] = up | gate
            w1c[g] = (
                s.reshape(KB1, 128, 2 * ISL).transpose(1, 0, 2).reshape(128, -1)
            )
            w2c[g] = (
                w2b[g][lo : lo + ISL]
                .reshape(KB2, 128, HIDDEN)
                .transpose(1, 0, 2)
                .reshape(128, -1)
            )
        in_maps.append({"xc": xc, "w1c": w1c, "w2c": w2c})

    kwargs = {}
    if _trace:
        kwargs = dict(trace=True, trace_cores=list(range(N_CORES)))
    res = run_bass_kernel_spmd(nc, in_maps, core_ids=list(range(N_CORES)), **kwargs)
    _LAST_RESULTS["res"] = res

    # ---- gather: fp32-sum the 8 bf16 partials, then unpack chunk layout ----
    acc = np.zeros((128, MO * total), np.float32)
    for c in range(N_CORES):
        acc += res.results[c]["yc"].astype(np.float32)

    yp_full = np.empty((total, HIDDEN), np.float32)
    for off, nl, _ in chunks:
        blk = acc[:, MO * off : MO * (off + nl)].reshape(128, MO, nl)
        yp_full[off : off + nl] = blk.transpose(1, 0, 2).reshape(HIDDEN, nl).T

    if total == T:
        return np.ascontiguousarray(yp_full)
    out = np.empty((T, HIDDEN), np.float32)
    for g in range(GROUPS):
        cnt = int(counts[g])
        out[offs[g] : offs[g + 1]] = yp_full[poffs[g] : poffs[g] + cnt]
    return out


# revision 33
# speedup vs baseline: 1.0868x; 1.0868x over previous
"""Grouped MLP (MoE expert FFN) Bass kernel for 8 Trainium2 NeuronCores.

Problem: 4096 tokens sorted by expert (8 experts, uneven counts), per-expert
GLU MLP:  h = x @ w1[g]  (-> up|gate, 2*2048 cols);  a = silu(up)*gate;
y = a @ w2[g].

Sharding: 8-way tensor-parallel over the intermediate dim (perfectly
load-balanced for ANY token counts).  Core c holds, for every expert g:
  w1 slice  [1024, 512] = up cols [c*256,(c+1)*256) | gate cols same range
  w2 slice  [256, 1024] = rows [c*256,(c+1)*256)
Every core processes ALL tokens (in per-expert chunks; the chunk->expert
table is compile-time constant and identical on all cores -> one SPMD
program) and emits a bf16 partial y; the host sums the 8 partials in fp32.

All tensors are bf16 on device (1 PE cycle/row like f32r, but half the HBM
traffic); PSUM accumulation is fp32.  DMAs are few and large with
partition-major DRAM layouts (2-8KB contiguous lines) so each transfer
spreads across all 16 SDMA engines.
"""

import sys

try:  # concourse normally comes from the container's PYTHONPATH
    import concourse  # noqa: F401
except ImportError:  # pragma: no cover - fallback for stripped env
    for _p in (
        "/root/.axon_site",
        "/root/.axon_site/_ro/trn_rl_repo",
        "/root/.axon_site/_ro/pypackages",
        "/opt/trn_rl_repo",
    ):
        if _p not in sys.path:
            sys.path.append(_p)

from contextlib import ExitStack

import numpy as np

NUM_TOKENS = 4096
HIDDEN = 1024
INTER = 2048
GROUPS = 8
N_CORES = 8
ISL = INTER // N_CORES  # 256: per-core inter slice (per up/gate half)
KB1 = HIDDEN // 128  # 8 fc1 k-blocks
KB2 = ISL // 128  # 2 fc2 k-blocks
MO = HIDDEN // 128  # 8 fc2 out row-blocks

_PROGRAM_CACHE: dict = {}


def _chunks_from_counts(counts):
    """Per-expert token chunks (off, nl, g); nl <= 512, 128-aligned splits.

    Experts are ordered by descending token count so early compute covers
    the DMA time of the remaining weight loads (chunk offsets stay in the
    token-sorted coordinate system; only processing order changes).
    """
    chunks = []
    offs = np.zeros(len(counts) + 1, np.int64)
    offs[1:] = np.cumsum(counts)
    order = sorted(range(len(counts)), key=lambda g: -int(counts[g]))
    for gi, g in enumerate(order):
        cnt = int(counts[g])
        off = int(offs[g])
        seg = 0
        if gi == 0 and cnt >= 512:
            chunks.append((off, 256, g))
            seg = 256
        while seg < cnt:
            rem = cnt - seg
            if rem > 512:
                # balance the split: 768 -> 384+384, 640 -> 384+256
                n = -(-rem // 512)
                nl = min(512, ((rem // n + 127) // 128) * 128)
            else:
                nl = rem
            chunks.append((off + seg, nl, g))
            seg += nl
    return tuple(chunks)


def _build_program(chunks, total):
    """Build + compile the single-core Bass program (same NEFF on all cores)."""
    import concourse.bass as bass  # noqa: F401
    import concourse.mybir as mybir
    import concourse.tile as tile
    from concourse import bacc

    f32 = mybir.dt.float32
    bf16 = mybir.dt.bfloat16
    silu = mybir.ActivationFunctionType.Silu

    nc = bacc.Bacc("TRN2", target_bir_lowering=False, debug=False)

    xc_d = nc.dram_tensor("xc", [128, KB1 * total], bf16, kind="ExternalInput").ap()
    w1_d = nc.dram_tensor(
        "w1c", [GROUPS, 128, KB1 * 2 * ISL], bf16, kind="ExternalInput"
    ).ap()
    w2_d = nc.dram_tensor(
        "w2c", [GROUPS, 128, KB2 * HIDDEN], bf16, kind="ExternalInput"
    ).ap()
    y_d = nc.dram_tensor("yc", [128, MO * total], bf16, kind="ExternalOutput").ap()

    expert_order = []
    for _, _, g in chunks:
        if g not in expert_order:
            expert_order.append(g)

    with tile.TileContext(nc) as tc, ExitStack() as ctx:
        wp = ctx.enter_context(tc.tile_pool(name="w", bufs=1))
        xp = ctx.enter_context(tc.tile_pool(name="x", bufs=6))
        hp = ctx.enter_context(tc.tile_pool(name="h", bufs=3))
        yp = ctx.enter_context(tc.tile_pool(name="y", bufs=3))
        tp = ctx.enter_context(tc.tile_pool(name="t", bufs=3))
        pup = ctx.enter_context(tc.tile_pool(name="pu", bufs=3, space="PSUM"))
        pgp = ctx.enter_context(tc.tile_pool(name="pg", bufs=2, space="PSUM"))
        pyp = ctx.enter_context(tc.tile_pool(name="py", bufs=3, space="PSUM"))

        # Warm-up matmuls: keep the PE busy from t=0 so the p-state ramps to
        # 2.4GHz while the first weight/token DMAs are still in flight.
        # Two alternating PSUM tiles so consecutive warm-ups pipeline.
        wu = wp.tile([128, 128], bf16, tag="warm")
        nc.gpsimd.memset(wu, 0)
        pwa = pyp.tile([128, 128], f32, tag="py", name="pwa")
        pwb = pyp.tile([128, 128], f32, tag="py", name="pwb")
        for i in range(112):
            nc.tensor.matmul(
                pwa if i % 2 == 0 else pwb, wu, wu, start=True, stop=True
            )

        # All weight loads go upfront on the Act HWDGE queue (they are never
        # slot-gated); token/output traffic runs on the SP queue so the two
        # streams cannot block each other.
        w1t = {}
        w2t = {}

        def load_weights(g):
            w1t[g] = wp.tile(
                [128, KB1 * 2 * ISL], bf16, tag=f"w1g{g}", name=f"w1t{g}"
            )
            nc.scalar.dma_start(out=w1t[g], in_=w1_d[g])
            w2t[g] = wp.tile(
                [128, KB2 * HIDDEN], bf16, tag=f"w2g{g}", name=f"w2t{g}"
            )
            nc.scalar.dma_start(out=w2t[g], in_=w2_d[g])

        load_weights(expert_order[0])

        def emit_fc2(off, nl, g, ht, final=False):
            """fc2 + psum->sbuf downcast + y store for a finished fc1 chunk."""
            yt = yp.tile([128, MO * nl], bf16, tag="y", name="yt")
            for mo in range(MO):
                py = pyp.tile([128, nl], f32, tag="py", name="py")
                for kb in range(KB2):
                    nc.tensor.matmul(
                        py,
                        w2t[g][:, kb * HIDDEN + mo * 128 : kb * HIDDEN + mo * 128 + 128],
                        ht[:, kb * nl : kb * nl + nl],
                        start=(kb == 0),
                        stop=(kb == KB2 - 1),
                    )
                dst = yt[:, mo * nl : mo * nl + nl]
                if mo % 2 == 0:
                    nc.scalar.copy(dst, py)
                else:
                    nc.vector.tensor_copy(dst, py)
                if final and mo == 3:
                    # first half leaves while mo 4-7 are still being copied
                    nc.sync.dma_start(
                        out=y_d[:, MO * off : MO * off + 4 * nl],
                        in_=yt[:, : 4 * nl],
                    )
            if final:
                nc.sync.dma_start(
                    out=y_d[:, MO * off + 4 * nl : MO * off + MO * nl],
                    in_=yt[:, 4 * nl :],
                )
            else:
                nc.sync.dma_start(
                    out=y_d[:, MO * off : MO * off + MO * nl], in_=yt
                )

        # Software-pipelined by one chunk: fc2 of chunk c-1 is emitted after
        # fc1 of chunk c, so fc2 never waits on the silu/mul chain and the
        # next chunk's silu precedes the previous chunk's copies on ACT.
        pending = None
        for off, nl, g in chunks:
            xt = xp.tile([128, KB1 * nl], bf16, tag="x")
            nc.sync.dma_start(
                out=xt, in_=xc_d[:, KB1 * off : KB1 * off + KB1 * nl]
            )
            # prefetch the next expert's weights one expert ahead; spreading
            # the 667ns ACT-sequencer issue slots across chunks keeps them
            # from ever blocking a pending silu
            gi = expert_order.index(g)
            if gi + 1 < len(expert_order) and expert_order[gi + 1] not in w1t:
                load_weights(expert_order[gi + 1])
            ht = hp.tile([128, KB2 * nl], bf16, tag="h")
            for i2 in range(KB2):
                pu = pup.tile([128, nl], f32, tag="pu")
                pg = pgp.tile([128, nl], f32, tag="pg")
                for kb in range(KB1):
                    rhs = xt[:, kb * nl : kb * nl + nl]
                    w = w1t[g][:, kb * 2 * ISL + i2 * 128 :]
                    nc.tensor.matmul(
                        pu, w[:, :128], rhs, start=(kb == 0), stop=(kb == KB1 - 1)
                    )
                for kb in range(KB1):
                    rhs = xt[:, kb * nl : kb * nl + nl]
                    w = w1t[g][:, kb * 2 * ISL + ISL + i2 * 128 :]
                    nc.tensor.matmul(
                        pg, w[:, :128], rhs, start=(kb == 0), stop=(kb == KB1 - 1)
                    )
                tmp = tp.tile([128, nl], f32, tag="tmp")
                nc.scalar.activation(tmp, pu, silu)
                nc.vector.tensor_mul(ht[:, i2 * nl : i2 * nl + nl], tmp, pg)
            if pending is not None:
                emit_fc2(*pending)
            pending = (off, nl, g, ht)
        emit_fc2(*pending, final=True)

    nc.compile()
    return nc


def _get_program(chunks, total):
    key = (chunks, total)
    if key not in _PROGRAM_CACHE:
        _PROGRAM_CACHE[key] = _build_program(chunks, total)
    return _PROGRAM_CACHE[key]


_LAST_RESULTS = {}  # exposed for test.py (exec time, trace paths)


def kernel(permuted_tokens, tokens_per_expert, w1, w2, _trace=False):
    import ml_dtypes
    from concourse.bass_utils import run_bass_kernel_spmd

    bf16 = ml_dtypes.bfloat16

    x = np.asarray(permuted_tokens, np.float32)
    counts = np.asarray(tokens_per_expert, np.int64)
    w1 = np.asarray(w1, np.float32)
    w2 = np.asarray(w2, np.float32)
    T = x.shape[0]

    offs = np.zeros(GROUPS + 1, np.int64)
    offs[1:] = np.cumsum(counts)

    # pad each expert segment to a multiple of 128 (no-op for 128-aligned counts)
    pcounts = [(-(-int(c) // 128)) * 128 for c in counts]
    total = sum(pcounts)
    poffs = np.zeros(GROUPS + 1, np.int64)
    poffs[1:] = np.cumsum(pcounts)
    if total == T and all(int(a) == b for a, b in zip(counts, pcounts)):
        xp = x
    else:
        xp = np.zeros((total, HIDDEN), np.float32)
        for g in range(GROUPS):
            xp[poffs[g] : poffs[g] + int(counts[g])] = x[offs[g] : offs[g + 1]]

    chunks = _chunks_from_counts(pcounts)
    nc = _get_program(chunks, total)

    # ---- host-side packing (shared token tensor; per-core weight slices) ----
    xcast = xp.astype(bf16)
    xc = np.empty((128, KB1 * total), bf16)
    for off, nl, _ in chunks:
        seg = xcast[off : off + nl].T.reshape(KB1, 128, nl).transpose(1, 0, 2)
        xc[:, KB1 * off : KB1 * (off + nl)] = seg.reshape(128, KB1 * nl)

    w1b = w1.astype(bf16)
    w2b = w2.astype(bf16)

    in_maps = []
    for c in range(N_CORES):
        w1c = np.empty((GROUPS, 128, KB1 * 2 * ISL), bf16)
        w2c = np.empty((GROUPS, 128, KB2 * HIDDEN), bf16)
        lo = c * ISL
        for g in range(GROUPS):
            s = np.concatenate(
                [w1b[g][:, lo : lo + ISL], w1b[g][:, INTER + lo : INTER + lo + ISL]],
                axis=1,
            )  # [1024, 512] = up | gate
            w1c[g] = (
                s.reshape(KB1, 128, 2 * ISL).transpose(1, 0, 2).reshape(128, -1)
            )
            w2c[g] = (
                w2b[g][lo : lo + ISL]
                .reshape(KB2, 128, HIDDEN)
                .transpose(1, 0, 2)
                .reshape(128, -1)
            )
        in_maps.append({"xc": xc, "w1c": w1c, "w2c": w2c})

    kwargs = {}
    if _trace:
        kwargs = dict(trace=True, trace_cores=list(range(N_CORES)))
    res = run_bass_kernel_spmd(nc, in_maps, core_ids=list(range(N_CORES)), **kwargs)
    _LAST_RESULTS["res"] = res

    # ---- gather: fp32-sum the 8 bf16 partials, then unpack chunk layout ----
    acc = np.zeros((128, MO * total), np.float32)
    for c in range(N_CORES):
        acc += res.results[c]["yc"].astype(np.float32)

    yp_full = np.empty((total, HIDDEN), np.float32)
    for off, nl, _ in chunks:
        blk = acc[:, MO * off : MO * (off + nl)].reshape(128, MO, nl)
        yp_full[off : off + nl] = blk.transpose(1, 0, 2).reshape(HIDDEN, nl).T

    if total == T:
        return np.ascontiguousarray(yp_full)
    out = np.empty((T, HIDDEN), np.float32)
    for g in range(GROUPS):
        cnt = int(counts[g])
        out[offs[g] : offs[g + 1]] = yp_full[poffs[g] : poffs[g] + cnt]
    return out
